# revision 39
# baseline (speedup 1.0000x reference)
"""AdvancedVectorMemory fused kernel for 8 Trainium2 NeuronCores.

Sharding: core c handles batch b = c//4 and heads 4*(c%4) .. 4*(c%4)+3
(data parallel over batch, tensor parallel over heads). Attention runs
flash-style per head pair with fused denominators (ones column in V).

Perf structure:
 - s-rotation: core (b, g) processes logical s-slices in the order
   g+1, g+2, g+3, g (mod 4), host-side permutation of q columns. Its
   own slice is computed LAST, so only 3 AllGathers are needed (the
   4th would carry data nobody else reads); each AG overlaps the next
   chunk pair's compute and the receive pipeline (gather + reciprocal
   of softmax denominators) runs during attention.
 - Wo / Wg1 input-channel blocks are host-permuted per core into
   gather-arrival order, so the epilogue consumes chunks uniformly.
 - exp batched in [128,1536] psum strips to amortize ACT overhead.
 - AllGather payload is bf16 raw retrieved + denominator rows.
 - Wo/Wg1 weights stream to SBUF as bf16 and qs preloads during the
   attention phase (sync-queue DMAs behind each chunk + DVE converts).
 - Wg1 @ q runs at the head of the tail; Wg1 @ o half 0 is interleaved
   with the Wo accumulation one dt behind.
"""
import sys
import numpy as np

for _p in ('/opt/trn_rl_repo', '/root/.axon_site/_ro/trn_rl_repo'):
    if _p not in sys.path:
        sys.path.insert(0, _p)

B, S, M = 2, 2048, 4096
DM, DK = 1024, 768
H, Dh = 16, 64
NC = 8
GS = 4           # group size (cores per batch)
SC_W = 512       # s-chunk width
N_SC = S // SC_W
N_MT = M // 128  # 32 m-tiles
SSL = S // GS    # per-core s-slice for the epilogue (512)

# Schraudolph exp-by-bits on VectorE: int16 convert of
# x*(128/ln2) + (16256 - 5.51); the int16 bits ARE the bf16 pattern of
# ~exp(x) (+-3% error, renormalized away by the softmax denominator).
SCH_SCALE = 184.66496239727872
SCH_MAGIC = 16250.49

_PROG = None


def _build_program():
    from concourse import bacc, mybir, tile
    import concourse.bass as bass

    F32 = mybir.dt.float32
    F32R = mybir.dt.float32r
    BF16 = mybir.dt.bfloat16
    I16 = mybir.dt.int16
    AF = mybir.ActivationFunctionType
    ALU = mybir.AluOpType

    nc = bacc.Bacc('TRN2', target_bir_lowering=False, debug=False, num_devices=NC)

    def din(name, shape, dt=F32R):
        return nc.dram_tensor(name, shape, dt, kind='ExternalInput').ap()

    qT = din('qT', [DM, S])
    mkT = din('mkT', [DK, M])
    mvT = din('mvT', [DK, M])
    wqT = din('wqT', [DM, 256])
    wkT = din('wkT', [DK, 256])
    wvT = din('wvT', [DK, 256])
    woT = din('woT', [DM, DM], BF16)
    wg1T = din('wg1T', [2 * DM, DM], BF16)
    wg2T = din('wg2T', [DM, 2])
    qsT = din('qsT', [DM, SSL], F32)
    bc0 = din('bc0', [2, 128])        # row0 = ones (gate broadcast)
    bqv = din('bqv', [2, 128], F32)
    bkv = din('bkv', [2, 128], F32)
    bo2v = din('bo2v', [8, 128], F32)
    bg1v = din('bg1v', [8, 128], F32)
    bg2v = din('bg2v', [2, 1], F32)
    vones = nc.dram_tensor('vones', [128, 8], BF16, kind='ExternalInput').ap()
    gidx = nc.dram_tensor('gidx', [8, 128], mybir.dt.int32, kind='ExternalInput').ap()
    didx = nc.dram_tensor('didx', [4, 128], mybir.dt.int32, kind='ExternalInput').ap()
    sel4 = din('sel4', [128, 256], BF16)

    out_t = nc.dram_tensor('out_t', [DM, SSL], F32, kind='ExternalOutput').ap()

    with tile.TileContext(nc) as tc:
        with tc.tile_pool(name='consts', bufs=1) as consts, \
             tc.tile_pool(name='pre', bufs=1) as pre, \
             tc.tile_pool(name='stage', bufs=2) as stage, \
             tc.tile_pool(name='dram', bufs=1, space='DRAM') as dram:

            # ---------------- small constants ----------------
            bq_sb = consts.tile([128, 2], F32, tag='bq_sb')
            bk_sb = consts.tile([128, 2], F32, tag='bk_sb')
            for p in range(2):
                nc.gpsimd.dma_start(out=bq_sb[:, p:p + 1], in_=bqv[p:p + 1, :])
                nc.gpsimd.dma_start(out=bk_sb[:, p:p + 1], in_=bkv[p:p + 1, :])
            gidx_sb = []
            for kc in range(8):
                gt = consts.tile([128, 1], mybir.dt.int32, tag=f'gidx{kc}',
                                 name=f'gidx{kc}')
                nc.gpsimd.dma_start(out=gt[:], in_=gidx[kc:kc + 1, :])
                gidx_sb.append(gt)
            didx_sb = []
            for ci in range(4):
                dt_ = consts.tile([128, 1], mybir.dt.int32, tag=f'didx{ci}',
                                  name=f'didx{ci}')
                nc.gpsimd.dma_start(out=dt_[:], in_=didx[ci:ci + 1, :])
                didx_sb.append(dt_)
            sel4_sb = consts.tile([128, 256], BF16, tag='sel4_sb')
            nc.gpsimd.dma_start(out=sel4_sb[:], in_=sel4[:])
            bc0_sb = consts.tile([2, 128], F32R, tag='bc0_sb')
            nc.gpsimd.dma_start(out=bc0_sb[:], in_=bc0[:])
            bo2_sb = consts.tile([128, 8], F32, tag='bo2_sb')
            bg1_sb = consts.tile([128, 8], F32, tag='bg1_sb')
            for k in range(8):
                nc.gpsimd.dma_start(out=bo2_sb[:, k:k + 1], in_=bo2v[k:k + 1, :])
                nc.gpsimd.dma_start(out=bg1_sb[:, k:k + 1], in_=bg1v[k:k + 1, :])
            bg2_sb = consts.tile([2, 1], F32, tag='bg2_sb')
            nc.gpsimd.dma_start(out=bg2_sb[:], in_=bg2v[:])
            wg2_sb = consts.tile([128, 16], F32R, tag='wg2_sb')
            for k in range(8):
                nc.gpsimd.dma_start(out=wg2_sb[:, 2 * k:2 * (k + 1)],
                                    in_=wg2T[128 * k:128 * (k + 1), :])

            # epilogue tiles preloaded/converted during the attention phase
            wo_bf = pre.tile([128, 8 * DM], BF16, tag='wo_bf')
            wg1_bf = pre.tile([128, 16 * DM], BF16, tag='wg1_bf')
            qs_sb = pre.tile([128, 8 * SSL], F32, tag='qs_sb')
            qs_bf = pre.tile([128, 8 * SSL], BF16, tag='qs_bf')
            # gathered raw retrieved chunks + denominator reciprocals
            rawk = pre.tile([128, 8 * 512], BF16, tag='rawk')
            dgt_all = pre.tile([128, 4 * 512], BF16, tag='dgt_all')

            def preload_step(step):
                # 32 steps: wo chunks 0-7, wg1 chunks 8-23, qs slices 24-31.
                # DMAs issue from the sync queue AFTER each chunk's rt writes,
                # so they never starve the front-phase K/V stream.
                if step < 8:
                    kc = step
                    nc.sync.dma_start(out=wo_bf[:, DM * kc:DM * (kc + 1)],
                                      in_=woT[128 * kc:128 * (kc + 1), :])
                elif step < 24:
                    kc = step - 8
                    nc.sync.dma_start(out=wg1_bf[:, DM * kc:DM * (kc + 1)],
                                      in_=wg1T[128 * kc:128 * (kc + 1), :])
                else:
                    k = step - 24
                    nc.sync.dma_start(out=qs_sb[:, SSL * k:SSL * (k + 1)],
                                      in_=qsT[128 * k:128 * (k + 1), :])
                    nc.vector.tensor_copy(qs_bf[:, SSL * k:SSL * (k + 1)],
                                          qs_sb[:, SSL * k:SSL * (k + 1)])

            rt_in = [dram.tile([264, 512], BF16, tag=f'rt_in{i}',
                               name=f'rt_in{i}') for i in range(4)]
            rt_og = [dram.tile([2112, 512], BF16, tag=f'rt_og{i}',
                               name=f'rt_og{i}', addr_space='Shared')
                     for i in range(3)]

            def gather_ci(ci):
                # gather arrival chunk ci (both pairs) + its denominator rows.
                # ci<3 reads the AG output; ci=3 reads this core's own rt_in
                # rows (its own slice, computed last). gpsimd-only: async,
                # no vector-queue occupancy mid-attention.
                src = rt_og[ci] if ci < 3 else rt_in[3]
                for p in range(2):
                    kc = 2 * ci + p
                    nc.gpsimd.indirect_dma_start(
                        out=rawk[:, 512 * kc:512 * (kc + 1)], out_offset=None,
                        in_=src[:],
                        in_offset=bass.IndirectOffsetOnAxis(ap=gidx_sb[kc][:], axis=0))
                nc.gpsimd.indirect_dma_start(
                    out=dgt_all[:, 512 * ci:512 * (ci + 1)], out_offset=None,
                    in_=src[:],
                    in_offset=bass.IndirectOffsetOnAxis(ap=didx_sb[ci][:], axis=0))

            rdr_all = pre.tile([128, 4 * 512], BF16, tag='rdr_all')

            def recip_ci(ci):
                with nc.allow_low_precision(reason='denominators renormalize'):
                    nc.vector.reciprocal(rdr_all[:, 512 * ci:512 * (ci + 1)],
                                         dgt_all[:, 512 * ci:512 * (ci + 1)])

            with tc.tile_pool(name='proj', bufs=1) as proj:
                # ---------------- phase A: projections ----------------
                qt_pair = [proj.tile([128, S], BF16, tag=f'qt_pair{p}',
                                     name=f'qt_pair{p}') for p in range(2)]
                kt_pair = [proj.tile([128, M], BF16, tag=f'kt_pair{p}',
                                     name=f'kt_pair{p}') for p in range(2)]
                v_sb = [proj.tile([128, 264], BF16, tag=f'v_sb{mt}',
                                  name=f'v_sb{mt}') for mt in range(N_MT)]

                with tc.tile_pool(name='qw', bufs=1) as qw, \
                     tc.tile_pool(name='qin', bufs=3) as qin, \
                     tc.tile_pool(name='psA', bufs=1, space='PSUM') as psA:
                    wq_sb = qw.tile([128, 2048], F32R, tag='wq_sb')
                    # wq chunk 0 + q chunk 0 lead the sync queue so the PE
                    # starts as early as possible
                    nc.sync.dma_start(out=wq_sb[:, 0:256], in_=wqT[0:128, :])
                    qt_chs = []
                    for k in range(8):
                        qt_ch = qin.tile([128, S], F32R, tag='qt_ch',
                                         name=f'qt_ch{k}')
                        nc.sync.dma_start(out=qt_ch[:],
                                          in_=qT[128 * k:128 * (k + 1), :])
                        qt_chs.append(qt_ch)
                        if k == 0:
                            nc.sync.dma_start(
                                out=wq_sb[:, 256:2048].rearrange(
                                    'p (k c) -> p k c', k=7),
                                in_=wqT[128:1024, :].rearrange(
                                    '(k p) c -> p k c', p=128))
                    pqs = [psA.tile([128, 512], F32, tag=f'pq{i}', name=f'pq{i}')
                           for i in range(8)]
                    for k in range(8):
                        qt_ch = qt_chs[k]
                        for p in range(2):
                            for sc in range(N_SC):
                                nc.tensor.matmul(
                                    pqs[4 * p + sc][:],
                                    wq_sb[:, 256 * k + 128 * p:256 * k + 128 * (p + 1)],
                                    qt_ch[:, SC_W * sc:SC_W * (sc + 1)],
                                    start=(k == 0), stop=(k == 7))
                    for p in range(2):
                        for sc in range(N_SC):
                            nc.vector.tensor_scalar_add(
                                qt_pair[p][:, SC_W * sc:SC_W * (sc + 1)],
                                pqs[4 * p + sc][:], bq_sb[:, p:p + 1])

                # ---------------- K/V proj + attention ----------------
                # Pool stack is LIFO. psAV opens first; chunk pair 0 runs
                # with narrow [128,1024] exp strips (4 banks) so psK/psV can
                # coexist and K/V projection overlaps the first attention
                # chunks. After pair 0: close the KV pools, switch to wide
                # [128,1536] strips (6 banks) for pairs 1-7.
                ctx_psAV = tc.tile_pool(name='psAV', bufs=1, space='PSUM')
                psAV = ctx_psAV.__enter__()

                def chunk_pair(sc, apool, rtst, psQK, sw, strip_w):
                    for p in range(2):
                        accA = psAV.tile([66, 512], F32, tag='accA')
                        accB = psAV.tile([66, 512], F32, tag='accB')
                        acc = [accA, accB]
                        j = 0
                        si = 0
                        while j < 64:
                            gw = min(sw, 64 - j)
                            tQ = psQK.tile([128, strip_w], F32, tag='tQ')
                            for k in range(gw):
                                mt, h = (j + k) // 2, (j + k) % 2
                                nc.tensor.matmul(
                                    tQ[:, 512 * k:512 * (k + 1)],
                                    kt_pair[p][64 * h:64 * (h + 1),
                                               128 * mt:128 * (mt + 1)],
                                    qt_pair[p][64 * h:64 * (h + 1),
                                               SC_W * sc:SC_W * (sc + 1)],
                                    start=True, stop=True)
                            at = apool.tile([128, strip_w], BF16, tag='at')
                            # every third strip runs as a Schraudolph
                            # bit-trick on the otherwise-idle VectorE
                            if si % 3 == 2:
                                nc.vector.tensor_scalar(
                                    at[:, 0:512 * gw].bitcast(I16),
                                    tQ[:, 0:512 * gw],
                                    SCH_SCALE, SCH_MAGIC, ALU.mult, ALU.add)
                            else:
                                nc.scalar.activation(at[:, 0:512 * gw],
                                                     tQ[:, 0:512 * gw], AF.Exp)
                            for k in range(gw):
                                mt, h = (j + k) // 2, (j + k) % 2
                                nc.tensor.matmul(
                                    acc[h][:],
                                    v_sb[mt][:, 66 * (2 * p + h):66 * (2 * p + h) + 66],
                                    at[:, 512 * k:512 * (k + 1)],
                                    start=(mt == 0), stop=(mt == N_MT - 1))
                            j += gw
                            si += 1
                        for h in range(2):
                            rt_t = rtst.tile([66, 512], BF16, tag='rt_t',
                                             name=f'rt_t{sc}{p}{h}')
                            nc.vector.tensor_copy(rt_t[:], acc[h][0:66, :])
                            nc.sync.dma_start(
                                out=rt_in[sc][132 * p + 66 * h:
                                              132 * p + 66 * (h + 1), :],
                                in_=rt_t[:])
                    if sc < 3:
                        nc.gpsimd.collective_compute(
                            'AllGather', ALU.bypass,
                            replica_groups=[list(range(NC))],
                            ins=[rt_in[sc][:].opt()],
                            outs=[rt_og[sc][:].opt()])
                    for st_i in range(8 * sc, 8 * (sc + 1)):
                        preload_step(st_i)

                ctx_mkin = tc.tile_pool(name='mkin', bufs=2)
                mkin = ctx_mkin.__enter__()
                ctx_kvw = tc.tile_pool(name='kvw', bufs=1)
                kvw = ctx_kvw.__enter__()
                ctx_psK = tc.tile_pool(name='psK', bufs=1, space='PSUM')
                psK = ctx_psK.__enter__()
                ctx_psV = tc.tile_pool(name='psV', bufs=1, space='PSUM')
                psV = ctx_psV.__enter__()
                wk_sb = kvw.tile([128, 1536], F32R, tag='wk_sb')
                wv_sb = kvw.tile([128, 1536], F32R, tag='wv_sb')
                for k in range(6):
                    nc.sync.dma_start(out=wk_sb[:, 256 * k:256 * (k + 1)],
                                      in_=wkT[128 * k:128 * (k + 1), :])
                    nc.sync.dma_start(out=wv_sb[:, 256 * k:256 * (k + 1)],
                                      in_=wvT[128 * k:128 * (k + 1), :])
                for mc in range(8):  # m blocks of 512, k-chunks in half-tiles
                    mkb, mvb = [], []
                    for hf in range(2):
                        kb = mkin.tile([128, 1536], F32R, tag=f'mkb{hf}',
                                       name=f'mkb{mc}_{hf}')
                        vb = mkin.tile([128, 1536], F32R, tag=f'mvb{hf}',
                                       name=f'mvb{mc}_{hf}')
                        for kk in range(3):
                            k = 3 * hf + kk
                            nc.sync.dma_start(
                                out=kb[:, 512 * kk:512 * (kk + 1)],
                                in_=mkT[128 * k:128 * (k + 1), 512 * mc:512 * (mc + 1)])
                            nc.sync.dma_start(
                                out=vb[:, 512 * kk:512 * (kk + 1)],
                                in_=mvT[128 * k:128 * (k + 1), 512 * mc:512 * (mc + 1)])
                        mkb.append(kb)
                        mvb.append(vb)
                    for p in range(2):
                        pk = psK.tile([128, 512], F32, tag='pk')
                        for k in range(6):
                            nc.tensor.matmul(
                                pk[:],
                                wk_sb[:, 256 * k + 128 * p:256 * k + 128 * (p + 1)],
                                mkb[k // 3][:, 512 * (k % 3):512 * (k % 3 + 1)],
                                start=(k == 0), stop=(k == 5))
                        nc.vector.tensor_scalar_add(
                            kt_pair[p][:, 512 * mc:512 * (mc + 1)], pk[:],
                            bk_sb[:, p:p + 1])
                    for ml in range(4):
                        mt = 4 * mc + ml
                        pv = psV.tile([128, 256], F32, tag='pv')
                        for k in range(6):
                            nc.tensor.matmul(
                                pv[:],
                                mvb[k // 3][:, 512 * (k % 3) + 128 * ml:
                                            512 * (k % 3) + 128 * (ml + 1)],
                                wv_sb[:, 256 * k:256 * (k + 1)],
                                start=(k == 0), stop=(k == 5))
                        vh = v_sb[mt].rearrange('p (h c) -> p h c', h=4)
                        nc.sync.dma_start(
                            out=vh[:, :, 64:66],
                            in_=vones[:].rearrange('p (h c) -> p h c', h=4))
                        nc.vector.tensor_copy(
                            vh[:, :, 0:64],
                            pv[:].rearrange('p (h d) -> p h d', h=4))

                # chunk pair 0: narrow strips, overlapped with K/V projection
                ctx_attnA = tc.tile_pool(name='attnA', bufs=2)
                apoolA = ctx_attnA.__enter__()
                ctx_rtstA = tc.tile_pool(name='rtstA', bufs=2)
                rtstA = ctx_rtstA.__enter__()
                ctx_psQKA = tc.tile_pool(name='psQKA', bufs=2, space='PSUM')
                psQKA = ctx_psQKA.__enter__()
                chunk_pair(0, apoolA, rtstA, psQKA, 2, 1024)
                ctx_psQKA.__exit__(None, None, None)
                ctx_rtstA.__exit__(None, None, None)
                ctx_attnA.__exit__(None, None, None)
                ctx_psV.__exit__(None, None, None)
                ctx_psK.__exit__(None, None, None)
                ctx_kvw.__exit__(None, None, None)
                ctx_mkin.__exit__(None, None, None)

                # pairs 1-3: wide strips
                with tc.tile_pool(name='attnB', bufs=6) as apoolB, \
                     tc.tile_pool(name='rtstB', bufs=6) as rtstB, \
                     tc.tile_pool(name='psQKB', bufs=3, space='PSUM') as psQKB:
                    for sc in range(1, N_SC):
                        chunk_pair(sc, apoolB, rtstB, psQKB, 2, 1024)
                        if sc >= 2:
                            gather_ci(sc - 2)
                            recip_ci(sc - 2)
                    gather_ci(2)
                    recip_ci(2)
                ctx_psAV.__exit__(None, None, None)


            # ---------------- epilogue (own s-slice) ----------------
            with tc.tile_pool(name='ep', bufs=1) as ep, \
                 tc.tile_pool(name='ept', bufs=3) as ept:
                gq_sb = ep.tile([128, 8 * 512], F32, tag='gq_sb')
                # Wg1 @ q for both hidden halves — no AG dependency
                for half in range(2):
                    with tc.tile_pool(name=f'psGQ{half}', bufs=1,
                                      space='PSUM') as psGQ:
                        pgq = [psGQ.tile([128, 512], F32, tag=f'pgq{i}',
                                         name=f'pgq{half}_{i}') for i in range(4)]
                        for kc in range(8):
                            for i in range(4):
                                dt = 4 * half + i
                                nc.tensor.matmul(
                                    pgq[i][:],
                                    wg1_bf[:, DM * kc + 128 * dt:DM * kc + 128 * (dt + 1)],
                                    qs_bf[:, SSL * kc:SSL * (kc + 1)],
                                    start=(kc == 0), stop=(kc == 7))
                        for i in range(4):
                            dt = 4 * half + i
                            nc.vector.tensor_copy(
                                gq_sb[:, 512 * dt:512 * (dt + 1)], pgq[i][:])

                # own (last) chunk gather; its reciprocal runs on the
                # DVE while the PE does Wg1 @ q
                gather_ci(3)
                recip_ci(3)

                # normalize gathered chunks (bf16): reciprocal -> broadcast
                rtn = ep.tile([128, 8 * 512], BF16, tag='rtn')
                with tc.tile_pool(name='psN', bufs=2, space='PSUM') as psN:
                    for ci in range(4):
                        rdr = rdr_all[:, 512 * ci:512 * (ci + 1)]
                        for p in range(2):
                            kc = 2 * ci + p
                            bcp = psN.tile([128, 512], F32, tag='bcp')
                            nc.tensor.matmul(bcp[:],
                                             sel4_sb[:, 128 * p:128 * (p + 1)],
                                             rdr, start=True, stop=True)
                            nc.vector.tensor_tensor(
                                rtn[:, 512 * kc:512 * (kc + 1)],
                                rawk[:, 512 * kc:512 * (kc + 1)], bcp[:], ALU.mult)

                # Wo projection interleaved with Wg1@o half 0
                oT = ep.tile([128, 8 * SSL], BF16, tag='oT')
                sl = ep.tile([128, 8 * 512], F32R, tag='sl')

                def silu_block(pg, dt):
                    hg = ept.tile([128, 512], F32, tag='hg', name=f'hg{dt}')
                    nc.vector.tensor_tensor(
                        hg[:], pg[:], gq_sb[:, 512 * dt:512 * (dt + 1)], ALU.add)
                    sg = ept.tile([128, 512], F32, tag='sg', name=f'sg{dt}')
                    nc.scalar.activation(sg[:], hg[:], AF.Sigmoid,
                                         bias=bg1_sb[:, dt:dt + 1])
                    gg = ept.tile([128, 512], F32, tag='gg', name=f'gg{dt}')
                    nc.vector.tensor_scalar_add(gg[:], hg[:], bg1_sb[:, dt:dt + 1])
                    nc.vector.tensor_tensor(
                        sl[:, 512 * dt:512 * (dt + 1)], gg[:], sg[:], ALU.mult)

                def wg1o_step(pgo, half, dt, start, stop):
                    kc = 8 + dt
                    for i in range(4):
                        nc.tensor.matmul(
                            pgo[i][:],
                            wg1_bf[:, DM * kc + 512 * half + 128 * i:
                                   DM * kc + 512 * half + 128 * (i + 1)],
                            oT[:, SSL * dt:SSL * (dt + 1)],
                            start=start, stop=stop)

                ctx_psG0 = tc.tile_pool(name='psG0', bufs=1, space='PSUM')
                psG0 = ctx_psG0.__enter__()
                pgo0 = [psG0.tile([128, 512], F32, tag=f'pgo0_{i}',
                                  name=f'pgo0_{i}') for i in range(4)]
                ctx_psWo = tc.tile_pool(name='psWo', bufs=2, space='PSUM')
                psWo = ctx_psWo.__enter__()
                for dt in range(8):
                    po = psWo.tile([128, 512], F32, tag='po')
                    for kc in range(8):
                        nc.tensor.matmul(
                            po[:], wo_bf[:, DM * kc + 128 * dt:DM * kc + 128 * (dt + 1)],
                            rtn[:, 512 * kc:512 * (kc + 1)],
                            start=(kc == 0), stop=(kc == 7))
                    nc.vector.tensor_scalar_add(
                        oT[:, SSL * dt:SSL * (dt + 1)], po[:], bo2_sb[:, dt:dt + 1])
                    # Wg1 @ o (half 0) lags one dt so PE never waits on the
                    # DVE bias-add that materializes oT
                    if dt >= 1:
                        wg1o_step(pgo0, 0, dt - 1, start=(dt == 1), stop=False)
                ctx_psWo.__exit__(None, None, None)
                wg1o_step(pgo0, 0, 7, start=False, stop=True)
                for i in range(4):
                    silu_block(pgo0[i], i)
                ctx_psG0.__exit__(None, None, None)

                with tc.tile_pool(name='psG1', bufs=1, space='PSUM') as psG1:
                    pgo1 = [psG1.tile([128, 512], F32, tag=f'pgo1_{i}',
                                      name=f'pgo1_{i}') for i in range(4)]
                    for dt in range(8):
                        wg1o_step(pgo1, 1, dt, start=(dt == 0), stop=(dt == 7))
                    for i in range(4):
                        silu_block(pgo1[i], 4 + i)

                # gate scalar: sigmoid(Wg2 @ sl + bg2), broadcast to 128 rows
                with tc.tile_pool(name='psT', bufs=1, space='PSUM') as psT:
                    pgt = psT.tile([2, 512], F32, tag='pgt')
                    for kc in range(8):
                        nc.tensor.matmul(pgt[:], wg2_sb[:, 2 * kc:2 * (kc + 1)],
                                         sl[:, 512 * kc:512 * (kc + 1)],
                                         start=(kc == 0), stop=(kc == 7))
                    gate = ep.tile([2, 512], F32R, tag='gate')
                    nc.scalar.activation(gate[:], pgt[:], AF.Sigmoid, bias=bg2_sb[:])
                    gb = psT.tile([128, 512], F32, tag='gb')
                    nc.tensor.matmul(gb[:], bc0_sb[:], gate[:], start=True, stop=True)
                    gbs = ep.tile([128, 512], F32, tag='gbs')
                    nc.vector.tensor_copy(gbs[:], gb[:])

                    # out = q + gate * o
                    for dt in range(8):
                        go = ept.tile([128, 512], F32, tag='go')
                        nc.vector.tensor_tensor(
                            go[:], gbs[:], oT[:, SSL * dt:SSL * (dt + 1)], ALU.mult)
                        fo = ept.tile([128, 512], F32, tag='fo')
                        nc.vector.tensor_tensor(
                            fo[:], go[:], qs_sb[:, SSL * dt:SSL * (dt + 1)], ALU.add)
                        nc.sync.dma_start(out=out_t[128 * dt:128 * (dt + 1), :],
                                          in_=fo[:])

    nc.compile()
    return nc


def _shard(inputs):
    import ml_dtypes
    _bf16 = ml_dtypes.bfloat16
    q = np.asarray(inputs['query'], np.float32)
    mk = np.asarray(inputs['memory_keys'], np.float32)
    mv = np.asarray(inputs['memory_values'], np.float32)
    Wq = np.asarray(inputs['Wq'], np.float32); bq = np.asarray(inputs['bq'], np.float32)
    Wk = np.asarray(inputs['Wk'], np.float32); bk = np.asarray(inputs['bk'], np.float32)
    Wv = np.asarray(inputs['Wv'], np.float32); bv = np.asarray(inputs['bv'], np.float32)
    Wo = np.asarray(inputs['Wo'], np.float32); bo = np.asarray(inputs['bo'], np.float32)
    Wg1 = np.asarray(inputs['Wg1'], np.float32); bg1 = np.asarray(inputs['bg1'], np.float32)
    Wg2 = np.asarray(inputs['Wg2'], np.float32); bg2 = np.asarray(inputs['bg2'], np.float32)

    scale = Dh ** -0.5
    bo2 = bo + Wo @ bv
    bc0 = np.zeros((2, 128), np.float32)
    bc0[0, :] = 1.0
    wg2T = np.zeros((DM, 2), np.float32)
    wg2T[:, 0] = Wg2[0]
    bg2v = np.zeros((2, 1), np.float32)
    bg2v[:, 0] = bg2[0]
    # sel4[2p + j//64, 128p + j] = 1 — picks denominator-recip row 2p+h
    _sel4 = np.zeros((128, 256), np.float32)
    for _p in range(2):
        for _j in range(128):
            _sel4[2 * _p + _j // 64, 128 * _p + _j] = 1.0

    qT_b = [np.ascontiguousarray(q[b].T) for b in range(B)]
    mkT_b = [np.ascontiguousarray(mk[b].T) for b in range(B)]
    mvT_b = [np.ascontiguousarray(mv[b].T) for b in range(B)]
    WoT = np.ascontiguousarray(Wo.T)     # [1024 in, 1024 out]
    Wg1T = np.ascontiguousarray(Wg1.T)   # [2048 in, 1024 out]

    in_maps = []
    for c in range(NC):
        b, g = c // GS, c % GS
        hs = slice(64 * 4 * g, 64 * (4 * g + 4))  # rows of W for this core's 4 heads
        # s-rotation: compile chunk i processes logical slice (g+1+i)%4
        lsl = [(g + 1 + i) % 4 for i in range(4)]
        qT_c = np.concatenate([qT_b[b][:, 512 * l:512 * (l + 1)] for l in lsl],
                              axis=1)
        # arrival chunk ci comes from group-rank r_i = (g-1-ci)%4; its pair-p
        # block maps to Wo/Wg1 input-channel block 2*r_i + p
        ch = [2 * ((g - 1 - ci) % 4) + p for ci in range(4) for p in range(2)]
        woT_c = np.concatenate([WoT[128 * cb:128 * (cb + 1), :] for cb in ch])
        wg1T_c = np.concatenate(
            [Wg1T[0:1024, :]]
            + [Wg1T[1024 + 128 * cb:1024 + 128 * (cb + 1), :] for cb in ch])
        # gather row of (ci, p, head h, dim d):
        #   ci<3: 264*(4b + r_i) + 132p + 66h + d   in rt_og[ci]
        #   ci=3: 264*3        + 132p + 66h + d     in rt_in (own slice)
        def _base(ci, p):
            if ci < 3:
                return 264 * (4 * b + (g - 1 - ci) % 4) + 132 * p
            return 132 * p
        _gidx = np.asarray(
            [[_base(kc // 2, kc % 2) + 66 * (j // 64) + (j % 64)
              for j in range(128)] for kc in range(8)], np.int32)
        # denominator rows: j = 2p + h (4 valid); junk rows point at a
        # denominator row too (never zero, keeps 1/x finite)
        _didx = np.asarray(
            [[_base(ci, (j // 2) % 2) + 66 * (j % 2) + 64 if j < 4
              else _base(ci, 0) + 64 for j in range(128)]
             for ci in range(4)], np.int32)
        in_maps.append({
            'qT': np.ascontiguousarray(qT_c),
            'mkT': mkT_b[b],
            'mvT': mvT_b[b],
            'wqT': np.ascontiguousarray((Wq[hs] * scale).T),
            'wkT': np.ascontiguousarray(Wk[hs].T),
            'wvT': np.ascontiguousarray(Wv[hs].T),
            'woT': np.ascontiguousarray(woT_c.astype(_bf16)),
            'wg1T': np.ascontiguousarray(wg1T_c.astype(_bf16)),
            'wg2T': wg2T,
            'qsT': np.ascontiguousarray(q[b].T[:, SSL * g:SSL * (g + 1)]),
            'vones': np.ascontiguousarray(np.tile([1.0, 0.0], 4)[None, :].repeat(128, 0).astype(_bf16)),
            'bc0': bc0,
            'bqv': np.ascontiguousarray((bq[hs] * scale).reshape(2, 128)),
            'bkv': np.ascontiguousarray(bk[hs].reshape(2, 128)),
            'bo2v': np.ascontiguousarray(bo2.reshape(8, 128)),
            'bg1v': np.ascontiguousarray(bg1.reshape(8, 128)),
            'bg2v': bg2v,
            'gidx': _gidx,
            'didx': _didx,
            'sel4': np.ascontiguousarray(_sel4.astype(_bf16)),
        })
    return in_maps


def _run(inputs, trace=False):
    global _PROG
    from concourse.bass_utils import run_bass_kernel_spmd
    if _PROG is None:
        _PROG = _build_program()
    in_maps = _shard(inputs)
    res = run_bass_kernel_spmd(_PROG, in_maps, list(range(NC)), trace=trace)
    out = np.empty((B, S, DM), np.float32)
    for c in range(NC):
        b, g = c // GS, c % GS
        out[b, SSL * g:SSL * (g + 1), :] = res.results[c]['out_t'].T
    return out, res


def kernel(**inputs) -> np.ndarray:
    out, _ = _run(inputs, trace=False)
    return out



# revision 40
# speedup vs baseline: 1.0835x; 1.0835x over previous
"""AdvancedVectorMemory fused kernel for 8 Trainium2 NeuronCores.

Sharding: core c handles batch b = c//4 and heads 4*(c%4) .. 4*(c%4)+3
(data parallel over batch, tensor parallel over heads). Attention runs
flash-style per head pair with fused denominators (ones column in V).

Perf structure:
 - s-rotation: core (b, g) processes logical s-slices in the order
   g+1, g+2, g+3, g (mod 4), host-side permutation of q columns. Its
   own slice is computed LAST, so only 3 AllGathers are needed (the
   4th would carry data nobody else reads); each AG overlaps the next
   chunk pair's compute and the receive pipeline (gather + reciprocal
   of softmax denominators) runs during attention.
 - Wo / Wg1 input-channel blocks are host-permuted per core into
   gather-arrival order, so the epilogue consumes chunks uniformly.
 - exp batched in [128,1536] psum strips to amortize ACT overhead.
 - AllGather payload is bf16 raw retrieved + denominator rows.
 - Wo/Wg1 weights stream to SBUF as bf16 and qs preloads during the
   attention phase (sync-queue DMAs behind each chunk + DVE converts).
 - Wg1 @ q runs at the head of the tail; Wg1 @ o half 0 is interleaved
   with the Wo accumulation one dt behind.
"""
import sys
import numpy as np

for _p in ('/opt/trn_rl_repo', '/root/.axon_site/_ro/trn_rl_repo'):
    if _p not in sys.path:
        sys.path.insert(0, _p)

B, S, M = 2, 2048, 4096
DM, DK = 1024, 768
H, Dh = 16, 64
NC = 8
GS = 4           # group size (cores per batch)
SC_W = 512       # s-chunk width
N_SC = S // SC_W
N_MT = M // 128  # 32 m-tiles
SSL = S // GS    # per-core s-slice for the epilogue (512)

_PROG = None


def _build_program():
    from concourse import bacc, mybir, tile
    import concourse.bass as bass

    F32 = mybir.dt.float32
    F32R = mybir.dt.float32r
    BF16 = mybir.dt.bfloat16
    AF = mybir.ActivationFunctionType
    ALU = mybir.AluOpType

    nc = bacc.Bacc('TRN2', target_bir_lowering=False, debug=False, num_devices=NC)

    def din(name, shape, dt=F32R):
        return nc.dram_tensor(name, shape, dt, kind='ExternalInput').ap()

    qT = din('qT', [DM, S])
    mkT = din('mkT', [DK, M])
    mvT = din('mvT', [DK, M])
    wqT = din('wqT', [DM, 256])
    wkT = din('wkT', [DK, 256])
    wvT = din('wvT', [DK, 256])
    woT = din('woT', [DM, DM], BF16)
    wg1T = din('wg1T', [2 * DM, DM], BF16)
    wg2T = din('wg2T', [DM, 2])
    qsT = din('qsT', [DM, SSL], F32)
    bc0 = din('bc0', [2, 128])        # row0 = ones (gate broadcast)
    bqv = din('bqv', [2, 128], F32)
    bkv = din('bkv', [2, 128], F32)
    bo2v = din('bo2v', [8, 128], F32)
    bg1v = din('bg1v', [8, 128], F32)
    bg2v = din('bg2v', [2, 1], F32)
    vones = nc.dram_tensor('vones', [128, 8], BF16, kind='ExternalInput').ap()
    gidx = nc.dram_tensor('gidx', [8, 128], mybir.dt.int32, kind='ExternalInput').ap()
    didx = nc.dram_tensor('didx', [4, 128], mybir.dt.int32, kind='ExternalInput').ap()
    sel4 = din('sel4', [128, 256])

    out_t = nc.dram_tensor('out_t', [DM, SSL], F32, kind='ExternalOutput').ap()

    with tile.TileContext(nc) as tc:
        with tc.tile_pool(name='consts', bufs=1) as consts, \
             tc.tile_pool(name='pre', bufs=1) as pre, \
             tc.tile_pool(name='stage', bufs=2) as stage, \
             tc.tile_pool(name='dram', bufs=1, space='DRAM') as dram:

            # ---------------- small constants ----------------
            bq_sb = consts.tile([128, 2], F32, tag='bq_sb')
            bk_sb = consts.tile([128, 2], F32, tag='bk_sb')
            for p in range(2):
                nc.gpsimd.dma_start(out=bq_sb[:, p:p + 1], in_=bqv[p:p + 1, :])
                nc.gpsimd.dma_start(out=bk_sb[:, p:p + 1], in_=bkv[p:p + 1, :])
            gidx_sb = []
            for kc in range(8):
                gt = consts.tile([128, 1], mybir.dt.int32, tag=f'gidx{kc}',
                                 name=f'gidx{kc}')
                nc.gpsimd.dma_start(out=gt[:], in_=gidx[kc:kc + 1, :])
                gidx_sb.append(gt)
            didx_sb = []
            for ci in range(4):
                dt_ = consts.tile([128, 1], mybir.dt.int32, tag=f'didx{ci}',
                                  name=f'didx{ci}')
                nc.gpsimd.dma_start(out=dt_[:], in_=didx[ci:ci + 1, :])
                didx_sb.append(dt_)
            sel4_sb = consts.tile([128, 256], F32R, tag='sel4_sb')
            nc.gpsimd.dma_start(out=sel4_sb[:], in_=sel4[:])
            bc0_sb = consts.tile([2, 128], F32R, tag='bc0_sb')
            nc.gpsimd.dma_start(out=bc0_sb[:], in_=bc0[:])
            bo2_sb = consts.tile([128, 8], F32, tag='bo2_sb')
            bg1_sb = consts.tile([128, 8], F32, tag='bg1_sb')
            for k in range(8):
                nc.gpsimd.dma_start(out=bo2_sb[:, k:k + 1], in_=bo2v[k:k + 1, :])
                nc.gpsimd.dma_start(out=bg1_sb[:, k:k + 1], in_=bg1v[k:k + 1, :])
            bg2_sb = consts.tile([2, 1], F32, tag='bg2_sb')
            nc.gpsimd.dma_start(out=bg2_sb[:], in_=bg2v[:])
            wg2_sb = consts.tile([128, 16], F32R, tag='wg2_sb')
            for k in range(8):
                nc.gpsimd.dma_start(out=wg2_sb[:, 2 * k:2 * (k + 1)],
                                    in_=wg2T[128 * k:128 * (k + 1), :])

            # epilogue tiles preloaded/converted during the attention phase
            wo_bf = pre.tile([128, 8 * DM], BF16, tag='wo_bf')
            wg1_bf = pre.tile([128, 16 * DM], BF16, tag='wg1_bf')
            qs_sb = pre.tile([128, 8 * SSL], F32, tag='qs_sb')
            qs_bf = pre.tile([128, 8 * SSL], BF16, tag='qs_bf')
            # gathered raw retrieved chunks + denominator reciprocals
            rawk = pre.tile([128, 8 * 512], BF16, tag='rawk')
            dgt_all = pre.tile([128, 4 * 512], BF16, tag='dgt_all')

            def preload_step(step):
                # 32 steps: wo chunks 0-7, wg1 chunks 8-23, qs slices 24-31.
                # DMAs issue from the sync queue AFTER each chunk's rt writes,
                # so they never starve the front-phase K/V stream.
                if step < 8:
                    kc = step
                    nc.sync.dma_start(out=wo_bf[:, DM * kc:DM * (kc + 1)],
                                      in_=woT[128 * kc:128 * (kc + 1), :])
                elif step < 24:
                    kc = step - 8
                    nc.sync.dma_start(out=wg1_bf[:, DM * kc:DM * (kc + 1)],
                                      in_=wg1T[128 * kc:128 * (kc + 1), :])
                else:
                    k = step - 24
                    nc.sync.dma_start(out=qs_sb[:, SSL * k:SSL * (k + 1)],
                                      in_=qsT[128 * k:128 * (k + 1), :])
                    nc.vector.tensor_copy(qs_bf[:, SSL * k:SSL * (k + 1)],
                                          qs_sb[:, SSL * k:SSL * (k + 1)])

            rt_in = [dram.tile([264, 512], BF16, tag=f'rt_in{i}',
                               name=f'rt_in{i}') for i in range(4)]
            rt_og = [dram.tile([2112, 512], BF16, tag=f'rt_og{i}',
                               name=f'rt_og{i}', addr_space='Shared')
                     for i in range(3)]

            def gather_ci(ci):
                # gather arrival chunk ci (both pairs) + its denominator rows.
                # ci<3 reads the AG output; ci=3 reads this core's own rt_in
                # rows (its own slice, computed last). gpsimd-only: async,
                # no vector-queue occupancy mid-attention.
                src = rt_og[ci] if ci < 3 else rt_in[3]
                for p in range(2):
                    kc = 2 * ci + p
                    nc.gpsimd.indirect_dma_start(
                        out=rawk[:, 512 * kc:512 * (kc + 1)], out_offset=None,
                        in_=src[:],
                        in_offset=bass.IndirectOffsetOnAxis(ap=gidx_sb[kc][:], axis=0))
                nc.gpsimd.indirect_dma_start(
                    out=dgt_all[:, 512 * ci:512 * (ci + 1)], out_offset=None,
                    in_=src[:],
                    in_offset=bass.IndirectOffsetOnAxis(ap=didx_sb[ci][:], axis=0))

            def recip_ci(ci, dpool):
                rdf = dpool.tile([128, 512], F32, tag='rdf', name=f'rdf{ci}')
                nc.vector.reciprocal(rdf[:], dgt_all[:, 512 * ci:512 * (ci + 1)])
                rdr = dpool.tile([128, 512], F32R, tag='rdr', name=f'rdr{ci}')
                nc.vector.tensor_copy(rdr[:], rdf[:])
                return rdr

            with tc.tile_pool(name='proj', bufs=1) as proj:
                # ---------------- phase A: projections ----------------
                qt_pair = [proj.tile([128, S], BF16, tag=f'qt_pair{p}',
                                     name=f'qt_pair{p}') for p in range(2)]
                kt_pair = [proj.tile([128, M], BF16, tag=f'kt_pair{p}',
                                     name=f'kt_pair{p}') for p in range(2)]
                v_sb = [proj.tile([128, 264], BF16, tag=f'v_sb{mt}',
                                  name=f'v_sb{mt}') for mt in range(N_MT)]

                with tc.tile_pool(name='qw', bufs=1) as qw, \
                     tc.tile_pool(name='qin', bufs=2) as qin, \
                     tc.tile_pool(name='psA', bufs=1, space='PSUM') as psA:
                    wq_sb = qw.tile([128, 2048], F32R, tag='wq_sb')
                    for k in range(8):
                        nc.sync.dma_start(out=wq_sb[:, 256 * k:256 * (k + 1)],
                                          in_=wqT[128 * k:128 * (k + 1), :])
                    pqs = [psA.tile([128, 512], F32, tag=f'pq{i}', name=f'pq{i}')
                           for i in range(8)]
                    for kg in range(4):  # q k-chunks stream in pairs
                        qt_ch = qin.tile([128, 2 * S], F32R, tag='qt_ch')
                        for kk in range(2):
                            k = 2 * kg + kk
                            nc.sync.dma_start(out=qt_ch[:, S * kk:S * (kk + 1)],
                                              in_=qT[128 * k:128 * (k + 1), :])
                        for kk in range(2):
                            k = 2 * kg + kk
                            for p in range(2):
                                for sc in range(N_SC):
                                    nc.tensor.matmul(
                                        pqs[4 * p + sc][:],
                                        wq_sb[:, 256 * k + 128 * p:256 * k + 128 * (p + 1)],
                                        qt_ch[:, S * kk + SC_W * sc:S * kk + SC_W * (sc + 1)],
                                        start=(k == 0), stop=(k == 7))
                    for p in range(2):
                        for sc in range(N_SC):
                            nc.vector.tensor_scalar_add(
                                qt_pair[p][:, SC_W * sc:SC_W * (sc + 1)],
                                pqs[4 * p + sc][:], bq_sb[:, p:p + 1])

                # ---------------- K/V proj + attention ----------------
                # Pool stack is LIFO. psAV opens first; chunk pair 0 runs
                # with narrow [128,1024] exp strips (4 banks) so psK/psV can
                # coexist and K/V projection overlaps the first attention
                # chunks. After pair 0: close the KV pools, switch to wide
                # [128,1536] strips (6 banks) for pairs 1-7.
                ctx_psAV = tc.tile_pool(name='psAV', bufs=1, space='PSUM')
                psAV = ctx_psAV.__enter__()

                def chunk_pair(sc, apool, rtst, psQK, sw, strip_w):
                    for p in range(2):
                        accA = psAV.tile([66, 512], F32, tag='accA')
                        accB = psAV.tile([66, 512], F32, tag='accB')
                        acc = [accA, accB]
                        j = 0
                        while j < 64:
                            gw = min(sw, 64 - j)
                            tQ = psQK.tile([128, strip_w], F32, tag='tQ')
                            for k in range(gw):
                                mt, h = (j + k) // 2, (j + k) % 2
                                nc.tensor.matmul(
                                    tQ[:, 512 * k:512 * (k + 1)],
                                    kt_pair[p][64 * h:64 * (h + 1),
                                               128 * mt:128 * (mt + 1)],
                                    qt_pair[p][64 * h:64 * (h + 1),
                                               SC_W * sc:SC_W * (sc + 1)],
                                    start=True, stop=True)
                            at = apool.tile([128, strip_w], BF16, tag='at')
                            nc.scalar.activation(at[:, 0:512 * gw],
                                                 tQ[:, 0:512 * gw], AF.Exp)
                            for k in range(gw):
                                mt, h = (j + k) // 2, (j + k) % 2
                                nc.tensor.matmul(
                                    acc[h][:],
                                    v_sb[mt][:, 66 * (2 * p + h):66 * (2 * p + h) + 66],
                                    at[:, 512 * k:512 * (k + 1)],
                                    start=(mt == 0), stop=(mt == N_MT - 1))
                            j += gw
                        for h in range(2):
                            rt_t = rtst.tile([66, 512], BF16, tag='rt_t',
                                             name=f'rt_t{sc}{p}{h}')
                            nc.vector.tensor_copy(rt_t[:], acc[h][0:66, :])
                            nc.sync.dma_start(
                                out=rt_in[sc][132 * p + 66 * h:
                                              132 * p + 66 * (h + 1), :],
                                in_=rt_t[:])
                    if sc < 3:
                        nc.gpsimd.collective_compute(
                            'AllGather', ALU.bypass,
                            replica_groups=[list(range(NC))],
                            ins=[rt_in[sc][:].opt()],
                            outs=[rt_og[sc][:].opt()])
                    for st_i in range(8 * sc, 8 * (sc + 1)):
                        preload_step(st_i)

                ctx_mkin = tc.tile_pool(name='mkin', bufs=2)
                mkin = ctx_mkin.__enter__()
                ctx_kvw = tc.tile_pool(name='kvw', bufs=1)
                kvw = ctx_kvw.__enter__()
                ctx_psK = tc.tile_pool(name='psK', bufs=1, space='PSUM')
                psK = ctx_psK.__enter__()
                ctx_psV = tc.tile_pool(name='psV', bufs=1, space='PSUM')
                psV = ctx_psV.__enter__()
                wk_sb = kvw.tile([128, 1536], F32R, tag='wk_sb')
                wv_sb = kvw.tile([128, 1536], F32R, tag='wv_sb')
                for k in range(6):
                    nc.sync.dma_start(out=wk_sb[:, 256 * k:256 * (k + 1)],
                                      in_=wkT[128 * k:128 * (k + 1), :])
                    nc.sync.dma_start(out=wv_sb[:, 256 * k:256 * (k + 1)],
                                      in_=wvT[128 * k:128 * (k + 1), :])
                for mc in range(8):  # m blocks of 512, k-chunks in half-tiles
                    mkb, mvb = [], []
                    for hf in range(2):
                        kb = mkin.tile([128, 1536], F32R, tag=f'mkb{hf}',
                                       name=f'mkb{mc}_{hf}')
                        vb = mkin.tile([128, 1536], F32R, tag=f'mvb{hf}',
                                       name=f'mvb{mc}_{hf}')
                        for kk in range(3):
                            k = 3 * hf + kk
                            nc.sync.dma_start(
                                out=kb[:, 512 * kk:512 * (kk + 1)],
                                in_=mkT[128 * k:128 * (k + 1), 512 * mc:512 * (mc + 1)])
                            nc.sync.dma_start(
                                out=vb[:, 512 * kk:512 * (kk + 1)],
                                in_=mvT[128 * k:128 * (k + 1), 512 * mc:512 * (mc + 1)])
                        mkb.append(kb)
                        mvb.append(vb)
                    for p in range(2):
                        pk = psK.tile([128, 512], F32, tag='pk')
                        for k in range(6):
                            nc.tensor.matmul(
                                pk[:],
                                wk_sb[:, 256 * k + 128 * p:256 * k + 128 * (p + 1)],
                                mkb[k // 3][:, 512 * (k % 3):512 * (k % 3 + 1)],
                                start=(k == 0), stop=(k == 5))
                        nc.vector.tensor_scalar_add(
                            kt_pair[p][:, 512 * mc:512 * (mc + 1)], pk[:],
                            bk_sb[:, p:p + 1])
                    for ml in range(4):
                        mt = 4 * mc + ml
                        pv = psV.tile([128, 256], F32, tag='pv')
                        for k in range(6):
                            nc.tensor.matmul(
                                pv[:],
                                mvb[k // 3][:, 512 * (k % 3) + 128 * ml:
                                            512 * (k % 3) + 128 * (ml + 1)],
                                wv_sb[:, 256 * k:256 * (k + 1)],
                                start=(k == 0), stop=(k == 5))
                        vh = v_sb[mt].rearrange('p (h c) -> p h c', h=4)
                        nc.sync.dma_start(
                            out=vh[:, :, 64:66],
                            in_=vones[:].rearrange('p (h c) -> p h c', h=4))
                        nc.vector.tensor_copy(
                            vh[:, :, 0:64],
                            pv[:].rearrange('p (h d) -> p h d', h=4))

                # chunk pair 0: narrow strips, overlapped with K/V projection
                ctx_attnA = tc.tile_pool(name='attnA', bufs=2)
                apoolA = ctx_attnA.__enter__()
                ctx_rtstA = tc.tile_pool(name='rtstA', bufs=2)
                rtstA = ctx_rtstA.__enter__()
                ctx_psQKA = tc.tile_pool(name='psQKA', bufs=2, space='PSUM')
                psQKA = ctx_psQKA.__enter__()
                chunk_pair(0, apoolA, rtstA, psQKA, 2, 1024)
                ctx_psQKA.__exit__(None, None, None)
                ctx_rtstA.__exit__(None, None, None)
                ctx_attnA.__exit__(None, None, None)
                ctx_psV.__exit__(None, None, None)
                ctx_psK.__exit__(None, None, None)
                ctx_kvw.__exit__(None, None, None)
                ctx_mkin.__exit__(None, None, None)

                # pairs 1-3: wide strips
                with tc.tile_pool(name='attnB', bufs=6) as apoolB, \
                     tc.tile_pool(name='rtstB', bufs=6) as rtstB, \
                     tc.tile_pool(name='psQKB', bufs=2, space='PSUM') as psQKB:
                    for sc in range(1, N_SC):
                        chunk_pair(sc, apoolB, rtstB, psQKB, 3, 1536)
                        if sc >= 2:
                            gather_ci(sc - 2)
                    gather_ci(2)
                ctx_psAV.__exit__(None, None, None)


            # ---------------- epilogue (own s-slice) ----------------
            with tc.tile_pool(name='ep', bufs=1) as ep, \
                 tc.tile_pool(name='ept', bufs=3) as ept:
                gq_sb = ep.tile([128, 8 * 512], F32, tag='gq_sb')
                # Wg1 @ q for both hidden halves — no AG dependency
                for half in range(2):
                    with tc.tile_pool(name=f'psGQ{half}', bufs=1,
                                      space='PSUM') as psGQ:
                        pgq = [psGQ.tile([128, 512], F32, tag=f'pgq{i}',
                                         name=f'pgq{half}_{i}') for i in range(4)]
                        for kc in range(8):
                            for i in range(4):
                                dt = 4 * half + i
                                nc.tensor.matmul(
                                    pgq[i][:],
                                    wg1_bf[:, DM * kc + 128 * dt:DM * kc + 128 * (dt + 1)],
                                    qs_bf[:, SSL * kc:SSL * (kc + 1)],
                                    start=(kc == 0), stop=(kc == 7))
                        for i in range(4):
                            dt = 4 * half + i
                            nc.vector.tensor_copy(
                                gq_sb[:, 512 * dt:512 * (dt + 1)], pgq[i][:])

                # own (last) chunk gather
                gather_ci(3)

                # normalize gathered chunks (bf16): reciprocal -> broadcast
                rtn = ep.tile([128, 8 * 512], BF16, tag='rtn')
                with tc.tile_pool(name='psN', bufs=2, space='PSUM') as psN:
                    for ci in range(4):
                        rdr = recip_ci(ci, ept)
                        for p in range(2):
                            kc = 2 * ci + p
                            bcp = psN.tile([128, 512], F32, tag='bcp')
                            nc.tensor.matmul(bcp[:],
                                             sel4_sb[:, 128 * p:128 * (p + 1)],
                                             rdr[:], start=True, stop=True)
                            nc.vector.tensor_tensor(
                                rtn[:, 512 * kc:512 * (kc + 1)],
                                rawk[:, 512 * kc:512 * (kc + 1)], bcp[:], ALU.mult)

                # Wo projection interleaved with Wg1@o half 0
                oT = ep.tile([128, 8 * SSL], BF16, tag='oT')
                sl = ep.tile([128, 8 * 512], F32R, tag='sl')

                def silu_block(pg, dt):
                    hg = ept.tile([128, 512], F32, tag='hg', name=f'hg{dt}')
                    nc.vector.tensor_tensor(
                        hg[:], pg[:], gq_sb[:, 512 * dt:512 * (dt + 1)], ALU.add)
                    sg = ept.tile([128, 512], F32, tag='sg', name=f'sg{dt}')
                    nc.scalar.activation(sg[:], hg[:], AF.Sigmoid,
                                         bias=bg1_sb[:, dt:dt + 1])
                    gg = ept.tile([128, 512], F32, tag='gg', name=f'gg{dt}')
                    nc.vector.tensor_scalar_add(gg[:], hg[:], bg1_sb[:, dt:dt + 1])
                    nc.vector.tensor_tensor(
                        sl[:, 512 * dt:512 * (dt + 1)], gg[:], sg[:], ALU.mult)

                def wg1o_step(pgo, half, dt, start, stop):
                    kc = 8 + dt
                    for i in range(4):
                        nc.tensor.matmul(
                            pgo[i][:],
                            wg1_bf[:, DM * kc + 512 * half + 128 * i:
                                   DM * kc + 512 * half + 128 * (i + 1)],
                            oT[:, SSL * dt:SSL * (dt + 1)],
                            start=start, stop=stop)

                ctx_psG0 = tc.tile_pool(name='psG0', bufs=1, space='PSUM')
                psG0 = ctx_psG0.__enter__()
                pgo0 = [psG0.tile([128, 512], F32, tag=f'pgo0_{i}',
                                  name=f'pgo0_{i}') for i in range(4)]
                ctx_psWo = tc.tile_pool(name='psWo', bufs=2, space='PSUM')
                psWo = ctx_psWo.__enter__()
                for dt in range(8):
                    po = psWo.tile([128, 512], F32, tag='po')
                    for kc in range(8):
                        nc.tensor.matmul(
                            po[:], wo_bf[:, DM * kc + 128 * dt:DM * kc + 128 * (dt + 1)],
                            rtn[:, 512 * kc:512 * (kc + 1)],
                            start=(kc == 0), stop=(kc == 7))
                    nc.vector.tensor_scalar_add(
                        oT[:, SSL * dt:SSL * (dt + 1)], po[:], bo2_sb[:, dt:dt + 1])
                    # Wg1 @ o (half 0) lags one dt so PE never waits on the
                    # DVE bias-add that materializes oT
                    if dt >= 1:
                        wg1o_step(pgo0, 0, dt - 1, start=(dt == 1), stop=False)
                ctx_psWo.__exit__(None, None, None)
                wg1o_step(pgo0, 0, 7, start=False, stop=True)
                for i in range(4):
                    silu_block(pgo0[i], i)
                ctx_psG0.__exit__(None, None, None)

                with tc.tile_pool(name='psG1', bufs=1, space='PSUM') as psG1:
                    pgo1 = [psG1.tile([128, 512], F32, tag=f'pgo1_{i}',
                                      name=f'pgo1_{i}') for i in range(4)]
                    for dt in range(8):
                        wg1o_step(pgo1, 1, dt, start=(dt == 0), stop=(dt == 7))
                    for i in range(4):
                        silu_block(pgo1[i], 4 + i)

                # gate scalar: sigmoid(Wg2 @ sl + bg2), broadcast to 128 rows
                with tc.tile_pool(name='psT', bufs=1, space='PSUM') as psT:
                    pgt = psT.tile([2, 512], F32, tag='pgt')
                    for kc in range(8):
                        nc.tensor.matmul(pgt[:], wg2_sb[:, 2 * kc:2 * (kc + 1)],
                                         sl[:, 512 * kc:512 * (kc + 1)],
                                         start=(kc == 0), stop=(kc == 7))
                    gate = ep.tile([2, 512], F32R, tag='gate')
                    nc.scalar.activation(gate[:], pgt[:], AF.Sigmoid, bias=bg2_sb[:])
                    gb = psT.tile([128, 512], F32, tag='gb')
                    nc.tensor.matmul(gb[:], bc0_sb[:], gate[:], start=True, stop=True)
                    gbs = ep.tile([128, 512], F32, tag='gbs')
                    nc.vector.tensor_copy(gbs[:], gb[:])

                    # out = q + gate * o
                    for dt in range(8):
                        go = ept.tile([128, 512], F32, tag='go')
                        nc.vector.tensor_tensor(
                            go[:], gbs[:], oT[:, SSL * dt:SSL * (dt + 1)], ALU.mult)
                        fo = ept.tile([128, 512], F32, tag='fo')
                        nc.vector.tensor_tensor(
                            fo[:], go[:], qs_sb[:, SSL * dt:SSL * (dt + 1)], ALU.add)
                        nc.sync.dma_start(out=out_t[128 * dt:128 * (dt + 1), :],
                                          in_=fo[:])

    nc.compile()
    return nc


def _shard(inputs):
    import ml_dtypes
    _bf16 = ml_dtypes.bfloat16
    q = np.asarray(inputs['query'], np.float32)
    mk = np.asarray(inputs['memory_keys'], np.float32)
    mv = np.asarray(inputs['memory_values'], np.float32)
    Wq = np.asarray(inputs['Wq'], np.float32); bq = np.asarray(inputs['bq'], np.float32)
    Wk = np.asarray(inputs['Wk'], np.float32); bk = np.asarray(inputs['bk'], np.float32)
    Wv = np.asarray(inputs['Wv'], np.float32); bv = np.asarray(inputs['bv'], np.float32)
    Wo = np.asarray(inputs['Wo'], np.float32); bo = np.asarray(inputs['bo'], np.float32)
    Wg1 = np.asarray(inputs['Wg1'], np.float32); bg1 = np.asarray(inputs['bg1'], np.float32)
    Wg2 = np.asarray(inputs['Wg2'], np.float32); bg2 = np.asarray(inputs['bg2'], np.float32)

    scale = Dh ** -0.5
    bo2 = bo + Wo @ bv
    bc0 = np.zeros((2, 128), np.float32)
    bc0[0, :] = 1.0
    wg2T = np.zeros((DM, 2), np.float32)
    wg2T[:, 0] = Wg2[0]
    bg2v = np.zeros((2, 1), np.float32)
    bg2v[:, 0] = bg2[0]
    # sel4[2p + j//64, 128p + j] = 1 — picks denominator-recip row 2p+h
    _sel4 = np.zeros((128, 256), np.float32)
    for _p in range(2):
        for _j in range(128):
            _sel4[2 * _p + _j // 64, 128 * _p + _j] = 1.0

    qT_b = [np.ascontiguousarray(q[b].T) for b in range(B)]
    mkT_b = [np.ascontiguousarray(mk[b].T) for b in range(B)]
    mvT_b = [np.ascontiguousarray(mv[b].T) for b in range(B)]
    WoT = np.ascontiguousarray(Wo.T)     # [1024 in, 1024 out]
    Wg1T = np.ascontiguousarray(Wg1.T)   # [2048 in, 1024 out]

    in_maps = []
    for c in range(NC):
        b, g = c // GS, c % GS
        hs = slice(64 * 4 * g, 64 * (4 * g + 4))  # rows of W for this core's 4 heads
        # s-rotation: compile chunk i processes logical slice (g+1+i)%4
        lsl = [(g + 1 + i) % 4 for i in range(4)]
        qT_c = np.concatenate([qT_b[b][:, 512 * l:512 * (l + 1)] for l in lsl],
                              axis=1)
        # arrival chunk ci comes from group-rank r_i = (g-1-ci)%4; its pair-p
        # block maps to Wo/Wg1 input-channel block 2*r_i + p
        ch = [2 * ((g - 1 - ci) % 4) + p for ci in range(4) for p in range(2)]
        woT_c = np.concatenate([WoT[128 * cb:128 * (cb + 1), :] for cb in ch])
        wg1T_c = np.concatenate(
            [Wg1T[0:1024, :]]
            + [Wg1T[1024 + 128 * cb:1024 + 128 * (cb + 1), :] for cb in ch])
        # gather row of (ci, p, head h, dim d):
        #   ci<3: 264*(4b + r_i) + 132p + 66h + d   in rt_og[ci]
        #   ci=3: 264*3        + 132p + 66h + d     in rt_in (own slice)
        def _base(ci, p):
            if ci < 3:
                return 264 * (4 * b + (g - 1 - ci) % 4) + 132 * p
            return 132 * p
        _gidx = np.asarray(
            [[_base(kc // 2, kc % 2) + 66 * (j // 64) + (j % 64)
              for j in range(128)] for kc in range(8)], np.int32)
        # denominator rows: j = 2p + h (4 valid); junk rows point at a
        # denominator row too (never zero, keeps 1/x finite)
        _didx = np.asarray(
            [[_base(ci, (j // 2) % 2) + 66 * (j % 2) + 64 if j < 4
              else _base(ci, 0) + 64 for j in range(128)]
             for ci in range(4)], np.int32)
        in_maps.append({
            'qT': np.ascontiguousarray(qT_c),
            'mkT': mkT_b[b],
            'mvT': mvT_b[b],
            'wqT': np.ascontiguousarray((Wq[hs] * scale).T),
            'wkT': np.ascontiguousarray(Wk[hs].T),
            'wvT': np.ascontiguousarray(Wv[hs].T),
            'woT': np.ascontiguousarray(woT_c.astype(_bf16)),
            'wg1T': np.ascontiguousarray(wg1T_c.astype(_bf16)),
            'wg2T': wg2T,
            'qsT': np.ascontiguousarray(q[b].T[:, SSL * g:SSL * (g + 1)]),
            'vones': np.ascontiguousarray(np.tile([1.0, 0.0], 4)[None, :].repeat(128, 0).astype(_bf16)),
            'bc0': bc0,
            'bqv': np.ascontiguousarray((bq[hs] * scale).reshape(2, 128)),
            'bkv': np.ascontiguousarray(bk[hs].reshape(2, 128)),
            'bo2v': np.ascontiguousarray(bo2.reshape(8, 128)),
            'bg1v': np.ascontiguousarray(bg1.reshape(8, 128)),
            'bg2v': bg2v,
            'gidx': _gidx,
            'didx': _didx,
            'sel4': _sel4,
        })
    return in_maps


def _run(inputs, trace=False):
    global _PROG
    from concourse.bass_utils import run_bass_kernel_spmd
    if _PROG is None:
        _PROG = _build_program()
    in_maps = _shard(inputs)
    res = run_bass_kernel_spmd(_PROG, in_maps, list(range(NC)), trace=trace)
    out = np.empty((B, S, DM), np.float32)
    for c in range(NC):
        b, g = c // GS, c % GS
        out[b, SSL * g:SSL * (g + 1), :] = res.results[c]['out_t'].T
    return out, res


def kernel(**inputs) -> np.ndarray:
    out, _ = _run(inputs, trace=False)
    return out

